# revision 7
# baseline (speedup 1.0000x reference)
"""Trainium2 Bass kernel for nn_DecoderModel_42228118454332.

Key algebraic structure of the reference model:
  - The 4-layer alignment MLP has no nonlinearities, so it composes into a
    single affine map e = x . m + c with m = W1^T W2^T W3^T W4^T.
  - x = [S | padded]; the S-dependent part of e is constant over encoder
    positions t, so it cancels inside softmax(axis=t). Attention weights
    therefore do not depend on the decoder state S at all.
  - The LSTM starts from zero state each step (w_hh sees h0=c0=0) and its
    input (the context) is step-invariant, so the output h is identical for
    all 50 decoder steps. The f-gate multiplies c0=0 and is never used.

Device computation per core k (SPMD over 8 cores):
  - a tiny AllGather at kernel start aligns the 8 cores (absorbs launch
    skew) while each core still has independent DMA/compute to do.
  - compose m_P = W1P^T @ (W2^T @ (W3^T @ W4^T)) on PE (only the `padded`
    1024:3072 slice of the input features matters).
  - batch shard: core k owns batches [8k, 8k+8). padTb = padded^T bf16
    feat-major, shipped in partition-major dump layout so every big DMA is
    128 large descriptors instead of thousands of small ones.
    e = m_P . padTb on PE -> [1, (b,t)]; softmax over t in fp32 on
    partition 0; broadcast a (bf16) to 128 partitions; context^T via
    multiply (bf16) + reduce (fp32 accum) split across DVE and GpSimd,
    in two halves.
  - AllGather each context half (bf16, [128, 64] dump layout); the L0
    matmuls on half 0 overlap half 1's reduce + collective. The gathered
    blocks are read back with one clean [128, 64] DMA per rank and fed to
    the PE through strided access patterns (no scatter DMAs).
  - LSTM tensor-sharded over the hidden dim: core k owns h rows
    [128k, 128(k+1)) of every layer, i.e. the matching i/g/o gate rows.
    gates = W_sel^T.T @ ct^T on PE (bf16 in, fp32 accum); sigmoid/tanh on
    ACT in fp32; AllGather h^T (bf16) between layers. Final layer writes
    the fp32 h^T slice out.
Host: concat slices -> h^T [1024, 64] -> h [64, 1024] -> broadcast to
  [50, 64, 1024].

Matmul operands are bf16: fp32 matmuls on TRN2 lower to HI/LO pairs (2x
instructions, ~4x PE time) and forgo fast weight load.
"""

import sys

for _p in ("/opt/trn_rl_repo", "/root/.axon_site/_ro/trn_rl_repo"):
    if _p not in sys.path:
        sys.path.insert(0, _p)

import ml_dtypes
import numpy as np

from concourse import bacc, mybir, tile
from concourse.bass_utils import run_bass_kernel_spmd

H = 1024          # hidden size
HH = 2 * H        # encoder feature size
T = 50            # encoder length == decoder steps
B = 64            # batch
AH = 256          # alignment hidden
NC = 8            # cores
BL = B // NC      # batches per core (8)
BT = BL * T       # 400
FO = HH // 128    # 16 feature chunks of padded part
FH = FO // 2      # 8 chunks per context half
KT1 = H // 128    # 8 k-tiles for layers 1..3

F32 = mybir.dt.float32
BF16 = mybir.dt.bfloat16
NPBF = ml_dtypes.bfloat16
RG = [list(range(NC))]

_CACHE = {}


def _pmajor(x, p=128):
    """[kt*p, cols] -> [p, kt*cols] partition-major dump layout."""
    kt = x.shape[0] // p
    return np.ascontiguousarray(
        x.reshape(kt, p, x.shape[1]).transpose(1, 0, 2).reshape(p, kt * x.shape[1])
    )


def _build():
    nc = bacc.Bacc("TRN2", target_bir_lowering=False, debug=False, num_devices=NC)

    # ---- kernel I/O (partition-major dump layouts) ----
    padTb = nc.dram_tensor("padTb", [128, FO * BT], BF16, kind="ExternalInput")
    w1p = nc.dram_tensor("w1p", [128, 2 * HH], BF16, kind="ExternalInput")
    w2 = nc.dram_tensor("w2", [128, 2 * AH], BF16, kind="ExternalInput")
    w3 = nc.dram_tensor("w3", [128, 2 * AH], BF16, kind="ExternalInput")
    w4t = nc.dram_tensor("w4t", [128, 2], BF16, kind="ExternalInput")
    wl = [
        nc.dram_tensor(
            f"wl{l}", [128, (FO if l == 0 else KT1) * 384], BF16, kind="ExternalInput"
        )
        for l in range(4)
    ]
    bih = [nc.dram_tensor(f"bih{l}", [128, 3], F32, kind="ExternalInput") for l in range(4)]
    bhh = [nc.dram_tensor(f"bhh{l}", [128, 3], F32, kind="ExternalInput") for l in range(4)]
    out = nc.dram_tensor("out", [128, B], F32, kind="ExternalOutput")

    # ---- collective internals (bf16) ----
    sk_in = nc.dram_tensor("sk_in", [128, 1], BF16)
    sk_ag = nc.dram_tensor("sk_ag", [128 * NC, 1], BF16, addr_space="Shared")
    ct_in = [nc.dram_tensor(f"ct_in{h}", [128, FH * BL], BF16) for h in range(2)]
    ct_ag = [
        nc.dram_tensor(f"ct_ag{h}", [128 * NC, FH * BL], BF16, addr_space="Shared")
        for h in range(2)
    ]
    h_in = [nc.dram_tensor(f"h_in{l}", [128, B], BF16) for l in range(3)]
    h_ag = [nc.dram_tensor(f"h_ag{l}", [H, B], BF16, addr_space="Shared") for l in range(3)]

    with tile.TileContext(nc) as tc:
        with (
            tc.tile_pool(name="big", bufs=1) as big,
            tc.tile_pool(name="work", bufs=3) as work,
            tc.tile_pool(name="psum", bufs=1, space="PSUM") as pp,
            tc.tile_pool(name="psg", bufs=4, space="PSUM") as pg,
        ):
            # ---------- rank-alignment barrier ----------
            sk_sb = work.tile([128, 1], BF16, tag="sk_sb")
            nc.gpsimd.memset(sk_sb[:], 0.0)
            nc.sync.dma_start(sk_in.ap(), sk_sb[:])
            nc.gpsimd.collective_compute(
                "AllGather", mybir.AluOpType.bypass, replica_groups=RG,
                ins=[sk_in.ap()], outs=[sk_ag.ap()],
            )

            # ---------- DMA loads ----------
            # critical path (sync ring): small align weights, then padTb
            w4t_sb = big.tile([128, 2, 1], BF16, tag="w4t_sb")
            nc.sync.dma_start(w4t_sb[:], w4t.ap().rearrange("p (kt o) -> p kt o", kt=2))
            w3_sb = big.tile([128, 2, AH], BF16, tag="w3_sb")
            nc.sync.dma_start(w3_sb[:], w3.ap().rearrange("p (kt j) -> p kt j", kt=2))
            w2_sb = big.tile([128, 2, AH], BF16, tag="w2_sb")
            nc.sync.dma_start(w2_sb[:], w2.ap().rearrange("p (kt j) -> p kt j", kt=2))
            w1p_sb = big.tile([128, 2, HH], BF16, tag="w1p_sb")
            nc.sync.dma_start(w1p_sb[:], w1p.ap().rearrange("p (kt j) -> p kt j", kt=2))

            padTb_sb = big.tile([128, FO, BT], BF16, tag="padTb_sb")
            N_PCHUNK = 4
            for c in range(N_PCHUNK):
                fo0 = c * (FO // N_PCHUNK)
                fo1 = (c + 1) * (FO // N_PCHUNK)
                nc.sync.dma_start(
                    padTb_sb[:, fo0:fo1, :],
                    padTb.ap()[:, fo0 * BT : fo1 * BT].rearrange(
                        "p (fo bt) -> p fo bt", bt=BT
                    ),
                )

            # weights on the scalar HWDGE ring (off the critical path)
            wl_sb = []
            for l in range(4):
                kt = FO if l == 0 else KT1
                t_ = big.tile([128, kt, 384], BF16, tag=f"wl{l}_sb")
                nc.scalar.dma_start(t_[:], wl[l].ap().rearrange("p (kt j) -> p kt j", j=384))
                wl_sb.append(t_)

            b_sb = []
            for l in range(4):
                t_i = work.tile([128, 3], F32, tag=f"bih{l}")
                nc.scalar.dma_start(t_i[:], bih[l].ap())
                t_h = work.tile([128, 3], F32, tag=f"bhh{l}")
                nc.scalar.dma_start(t_h[:], bhh[l].ap())
                t_b = big.tile([128, 3], F32, tag=f"b{l}")
                nc.vector.tensor_add(t_b[:], t_i[:], t_h[:])
                b_sb.append(t_b)

            # ---------- compose m_P on PE (bf16 in, fp32 accum) ----------
            ps_u = pp.tile([128, 2], F32, tag="ps_u")
            for mt in range(2):
                for kt in range(2):
                    nc.tensor.matmul(
                        ps_u[:, mt : mt + 1],
                        w3_sb[:, kt, mt * 128 : (mt + 1) * 128],
                        w4t_sb[:, kt, :],
                        start=(kt == 0),
                        stop=(kt == 1),
                    )
            u1_sb = big.tile([128, 2], BF16, tag="u1_sb")
            nc.vector.tensor_copy(u1_sb[:], ps_u[:])

            ps_u2 = pp.tile([128, 2], F32, tag="ps_u")
            for mt in range(2):
                for kt in range(2):
                    nc.tensor.matmul(
                        ps_u2[:, mt : mt + 1],
                        w2_sb[:, kt, mt * 128 : (mt + 1) * 128],
                        u1_sb[:, kt : kt + 1],
                        start=(kt == 0),
                        stop=(kt == 1),
                    )
            u2_sb = big.tile([128, 2], BF16, tag="u2_sb")
            nc.vector.tensor_copy(u2_sb[:], ps_u2[:])

            ps_m = pp.tile([128, FO], F32, tag="ps_m")
            for mt in range(FO):
                for kt in range(2):
                    nc.tensor.matmul(
                        ps_m[:, mt : mt + 1],
                        w1p_sb[:, kt, mt * 128 : (mt + 1) * 128],
                        u2_sb[:, kt : kt + 1],
                        start=(kt == 0),
                        stop=(kt == 1),
                    )
            m_sb = big.tile([128, FO], BF16, tag="m_sb")
            nc.vector.tensor_copy(m_sb[:], ps_m[:])

            # ---------- e = m_P . padTb  -> [1, (b, t)] ----------
            e_ps = pp.tile([1, BT], F32, tag="e_ps")
            for fo in range(FO):
                nc.tensor.matmul(
                    e_ps[:],
                    m_sb[:, fo : fo + 1],
                    padTb_sb[:, fo, :],
                    start=(fo == 0),
                    stop=(fo == FO - 1),
                )

            # ---------- softmax over t (partition 0, fp32) ----------
            e_sb = big.tile([1, BT], F32, tag="e_sb")
            nc.scalar.activation(e_sb[:], e_ps[:], mybir.ActivationFunctionType.Copy)
            e3 = e_sb[:].rearrange("p (b t) -> p b t", b=BL)
            mx = big.tile([1, BL], F32, tag="mx")
            nc.vector.reduce_max(mx[:], e3, axis=mybir.AxisListType.X)
            ec = big.tile([1, BT], F32, tag="ec")
            nc.vector.tensor_sub(
                ec[:].rearrange("p (b t) -> p b t", b=BL),
                e3,
                mx[:].unsqueeze(2).broadcast_to([1, BL, T]),
            )
            p_sb = big.tile([1, BT], F32, tag="p_sb")
            nc.scalar.activation(p_sb[:], ec[:], mybir.ActivationFunctionType.Exp)
            p3 = p_sb[:].rearrange("p (b t) -> p b t", b=BL)
            s_sb = big.tile([1, BL], F32, tag="s_sb")
            nc.vector.reduce_sum(s_sb[:], p3, axis=mybir.AxisListType.X)
            rs = big.tile([1, BL], F32, tag="rs")
            nc.vector.reciprocal(rs[:], s_sb[:])
            a_16 = big.tile([1, BT], BF16, tag="a_16")
            nc.vector.tensor_mul(
                a_16[:].rearrange("p (b t) -> p b t", b=BL),
                p3,
                rs[:].unsqueeze(2).broadcast_to([1, BL, T]),
            )
            a_bc = big.tile([128, BT], BF16, tag="a_bc")
            nc.gpsimd.partition_broadcast(a_bc[:], a_16[:])

            # ---------- context^T, two halves, DVE + GpSimd split ----------
            ctJ = []
            for hh in range(2):
                ctx_sb = work.tile([128, FH, BL], F32, tag="ctx_sb")
                for fi in range(FH):
                    fo = hh * FH + fi
                    eng = nc.gpsimd if fi >= 4 else nc.vector
                    prod = work.tile([128, BT], BF16, tag=f"prod{fi >= 4}")
                    eng.tensor_mul(prod[:], padTb_sb[:, fo, :], a_bc[:])
                    nc.vector.tensor_reduce(
                        ctx_sb[:, fi, :],
                        prod[:].rearrange("p (b t) -> p b t", b=BL),
                        op=mybir.AluOpType.add,
                        axis=mybir.AxisListType.X,
                    )
                ctb_sb = work.tile([128, FH, BL], BF16, tag="ctb_sb")
                nc.vector.tensor_copy(ctb_sb[:], ctx_sb[:])
                nc.sync.dma_start(ct_in[hh].ap(), ctb_sb[:].rearrange("p a b -> p (a b)"))
                nc.gpsimd.collective_compute(
                    "AllGather",
                    mybir.AluOpType.bypass,
                    replica_groups=RG,
                    ins=[ct_in[hh].ap()],
                    outs=[ct_ag[hh].ap()],
                )
                # one clean [128, 64] load per rank block
                ctj = big.tile([128, NC, FH, BL], BF16, tag=f"ctJ{hh}")
                ag_r = ct_ag[hh].ap().rearrange(
                    "(j p) fb -> j p fb", j=NC, p=128
                )
                for j in range(NC):
                    nc.sync.dma_start(
                        ctj[:, j, :, :],
                        ag_r[j].rearrange("p (fi b) -> p fi b", b=BL),
                    )
                ctJ.append(ctj)

            # ---------- 4-layer LSTM cell (i, g, o gates only) ----------
            rhs_sb = None  # layers >=1 use h tiles [128, kt, b]
            for l in range(4):
                kt_n = FO if l == 0 else KT1
                gates = []
                for m in range(3):
                    ps_g = pg.tile([128, B], F32, tag="gates")
                    for kt in range(kt_n):
                        if l == 0:
                            rhs = ctJ[kt // FH][:, :, kt % FH, :]
                        else:
                            rhs = rhs_sb[:, kt, :]
                        nc.tensor.matmul(
                            ps_g[:],
                            wl_sb[l][:, kt, m * 128 : (m + 1) * 128],
                            rhs,
                            start=(kt == 0),
                            stop=(kt == kt_n - 1),
                        )
                    gates.append(ps_g)
                sig_i = work.tile([128, B], F32, tag="sig_i")
                nc.scalar.activation(
                    sig_i[:], gates[0][:], mybir.ActivationFunctionType.Sigmoid,
                    bias=b_sb[l][:, 0:1],
                )
                tanh_g = work.tile([128, B], F32, tag="tanh_g")
                nc.scalar.activation(
                    tanh_g[:], gates[1][:], mybir.ActivationFunctionType.Tanh,
                    bias=b_sb[l][:, 1:2],
                )
                c_t = work.tile([128, B], F32, tag="c_t")
                nc.vector.tensor_mul(c_t[:], sig_i[:], tanh_g[:])
                tanh_c = work.tile([128, B], F32, tag="tanh_c")
                nc.scalar.activation(
                    tanh_c[:], c_t[:], mybir.ActivationFunctionType.Tanh
                )
                sig_o = work.tile([128, B], F32, tag="sig_o")
                nc.scalar.activation(
                    sig_o[:], gates[2][:], mybir.ActivationFunctionType.Sigmoid,
                    bias=b_sb[l][:, 2:3],
                )
                h_sl = work.tile([128, B], F32, tag="h_sl")
                nc.vector.tensor_mul(h_sl[:], sig_o[:], tanh_c[:])

                if l < 3:
                    h_slb = work.tile([128, B], BF16, tag="h_slb")
                    nc.vector.tensor_copy(h_slb[:], h_sl[:])
                    nc.sync.dma_start(h_in[l].ap(), h_slb[:])
                    nc.gpsimd.collective_compute(
                        "AllGather",
                        mybir.AluOpType.bypass,
                        replica_groups=RG,
                        ins=[h_in[l].ap()],
                        outs=[h_ag[l].ap()],
                    )
                    nxt = big.tile([128, KT1, B], BF16, tag=f"h{l}_sb")
                    nc.sync.dma_start(
                        nxt[:], h_ag[l].ap().rearrange("(kt p) b -> p kt b", p=128)
                    )
                    rhs_sb = nxt
                else:
                    nc.sync.dma_start(out.ap(), h_sl[:])

    nc.compile()
    return nc


def _prep_inputs(padded, align_ws, w_ih, b_ih, b_hh):
    """Build the 8 per-core input maps (host-side sharding / layout only)."""
    padded = np.asarray(padded, dtype=np.float32)
    w1 = np.asarray(align_ws[0], dtype=np.float32)
    shared = {
        "w1p": _pmajor(np.ascontiguousarray(w1[:, H:]).astype(NPBF)),
        "w2": _pmajor(np.asarray(align_ws[1], dtype=np.float32).astype(NPBF)),
        "w3": _pmajor(np.asarray(align_ws[2], dtype=np.float32).astype(NPBF)),
        "w4t": _pmajor(np.asarray(align_ws[3], dtype=np.float32).reshape(AH, 1).astype(NPBF)),
    }
    in_maps = []
    for k in range(NC):
        m = dict(shared)
        # padded^T for batches [8k, 8k+8): [2048, 8, 50] -> [2048, 400]
        sl = padded[:, k * BL : (k + 1) * BL, :]          # [50, 8, 2048]
        pT = np.ascontiguousarray(sl.transpose(2, 1, 0)).reshape(HH, BT)
        m["padTb"] = _pmajor(pT.astype(NPBF))
        for l in range(4):
            w = np.asarray(w_ih[l], dtype=np.float32)
            rows = np.concatenate(
                [w[g * H + k * 128 : g * H + (k + 1) * 128, :] for g in (0, 2, 3)],
                axis=0,
            )                                              # [384, fin] i,g,o rows
            m[f"wl{l}"] = _pmajor(np.ascontiguousarray(rows.T).astype(NPBF))
            for name, b in (("bih", b_ih[l]), ("bhh", b_hh[l])):
                bb = np.asarray(b, dtype=np.float32)
                bsel = np.stack(
                    [bb[g * H + k * 128 : g * H + (k + 1) * 128] for g in (0, 2, 3)],
                    axis=1,
                )                                          # [128, 3]
                m[f"{name}{l}"] = np.ascontiguousarray(bsel)
        in_maps.append(m)
    return in_maps


def kernel(padded, align_ws, align_bs, w_ih, w_hh, b_ih, b_hh):
    # align_bs and w_hh are mathematically inert: the MLP biases add a
    # t-invariant constant to e (cancels in softmax); w_hh multiplies the
    # zero initial LSTM state.
    if "nc" not in _CACHE:
        _CACHE["nc"] = _build()
    nc = _CACHE["nc"]
    in_maps = _prep_inputs(padded, align_ws, w_ih, b_ih, b_hh)
    res = run_bass_kernel_spmd(nc, in_maps, list(range(NC)))
    hT = np.concatenate([res.results[k]["out"] for k in range(NC)], axis=0)  # [1024, 64]
    h = hT.T                                                                  # [64, 1024]
    return np.ascontiguousarray(
        np.broadcast_to(h[None, :, :], (T, B, H))
    ).astype(np.float32)


# revision 8
# speedup vs baseline: 1.0534x; 1.0534x over previous
"""Trainium2 Bass kernel for nn_DecoderModel_42228118454332.

Key algebraic structure of the reference model:
  - The 4-layer alignment MLP has no nonlinearities, so it composes into a
    single affine map e = x . m + c with m = W1^T W2^T W3^T W4^T.
  - x = [S | padded]; the S-dependent part of e is constant over encoder
    positions t, so it cancels inside softmax(axis=t). Attention weights
    therefore do not depend on the decoder state S at all.
  - The LSTM starts from zero state each step (w_hh sees h0=c0=0) and its
    input (the context) is step-invariant, so the output h is identical for
    all 50 decoder steps. The f-gate multiplies c0=0 and is never used.

Device computation per core k (SPMD over 8 cores):
  - a tiny AllGather at kernel start aligns the 8 cores (absorbs launch
    skew) while each core still has independent DMA/compute to do.
  - compose m_P = W1P^T @ (W2^T @ (W3^T @ W4^T)) on PE (only the `padded`
    1024:3072 slice of the input features matters).
  - batch shard: core k owns batches [8k, 8k+8). padTb = padded^T bf16
    feat-major, shipped in partition-major dump layout so every big DMA is
    128 large descriptors instead of thousands of small ones.
    e = m_P . padTb on PE -> [1, (b,t)]; softmax over t in fp32 on
    partition 0; broadcast a (bf16) to 128 partitions; context^T via
    multiply (bf16) + reduce (fp32 accum) split across DVE and GpSimd,
    in two halves.
  - AllGather each context half (bf16, [128, 64] dump layout); the L0
    matmuls on half 0 overlap half 1's reduce + collective. The gathered
    blocks are read back with one clean [128, 64] DMA per rank and fed to
    the PE through strided access patterns (no scatter DMAs).
  - LSTM tensor-sharded over the hidden dim: core k owns h rows
    [128k, 128(k+1)) of every layer, i.e. the matching i/g/o gate rows.
    gates = W_sel^T.T @ ct^T on PE (bf16 in, fp32 accum); sigmoid/tanh on
    ACT in fp32; AllGather h^T (bf16) between layers. Final layer writes
    the fp32 h^T slice out.
Host: concat slices -> h^T [1024, 64] -> h [64, 1024] -> broadcast to
  [50, 64, 1024].

Matmul operands are bf16: fp32 matmuls on TRN2 lower to HI/LO pairs (2x
instructions, ~4x PE time) and forgo fast weight load.
"""

import sys

for _p in ("/opt/trn_rl_repo", "/root/.axon_site/_ro/trn_rl_repo"):
    if _p not in sys.path:
        sys.path.insert(0, _p)

import ml_dtypes
import numpy as np

from concourse import bacc, mybir, tile
from concourse.bass_utils import run_bass_kernel_spmd

H = 1024          # hidden size
HH = 2 * H        # encoder feature size
T = 50            # encoder length == decoder steps
B = 64            # batch
AH = 256          # alignment hidden
NC = 8            # cores
BL = B // NC      # batches per core (8)
BT = BL * T       # 400
FO = HH // 128    # 16 feature chunks of padded part
FH = FO // 2      # 8 chunks per context half
KT1 = H // 128    # 8 k-tiles for layers 1..3

F32 = mybir.dt.float32
BF16 = mybir.dt.bfloat16
NPBF = ml_dtypes.bfloat16
RG = [list(range(NC))]

_CACHE = {}


def _pmajor(x, p=128):
    """[kt*p, cols] -> [p, kt*cols] partition-major dump layout."""
    kt = x.shape[0] // p
    return np.ascontiguousarray(
        x.reshape(kt, p, x.shape[1]).transpose(1, 0, 2).reshape(p, kt * x.shape[1])
    )


def _build():
    nc = bacc.Bacc("TRN2", target_bir_lowering=False, debug=False, num_devices=NC)

    # ---- kernel I/O (partition-major dump layouts) ----
    padTb = nc.dram_tensor("padTb", [128, FO * BT], BF16, kind="ExternalInput")
    w1p = nc.dram_tensor("w1p", [128, 2 * HH], BF16, kind="ExternalInput")
    w2 = nc.dram_tensor("w2", [128, 2 * AH], BF16, kind="ExternalInput")
    w3 = nc.dram_tensor("w3", [128, 2 * AH], BF16, kind="ExternalInput")
    w4t = nc.dram_tensor("w4t", [128, 2], BF16, kind="ExternalInput")
    wl = [
        nc.dram_tensor(
            f"wl{l}", [128, (FO if l == 0 else KT1) * 384], BF16, kind="ExternalInput"
        )
        for l in range(4)
    ]
    bih = [nc.dram_tensor(f"bih{l}", [128, 3], F32, kind="ExternalInput") for l in range(4)]
    bhh = [nc.dram_tensor(f"bhh{l}", [128, 3], F32, kind="ExternalInput") for l in range(4)]
    out = nc.dram_tensor("out", [128, B], F32, kind="ExternalOutput")

    # ---- collective internals (bf16) ----
    ct_in = [nc.dram_tensor(f"ct_in{h}", [128, FH * BL], BF16) for h in range(2)]
    ct_ag = [
        nc.dram_tensor(f"ct_ag{h}", [128 * NC, FH * BL], BF16, addr_space="Shared")
        for h in range(2)
    ]
    h_in = [nc.dram_tensor(f"h_in{l}", [128, B], BF16) for l in range(3)]
    h_ag = [nc.dram_tensor(f"h_ag{l}", [H, B], BF16, addr_space="Shared") for l in range(3)]

    with tile.TileContext(nc) as tc:
        with (
            tc.tile_pool(name="big", bufs=1) as big,
            tc.tile_pool(name="work", bufs=3) as work,
            tc.tile_pool(name="psum", bufs=1, space="PSUM") as pp,
            tc.tile_pool(name="psg", bufs=4, space="PSUM") as pg,
        ):
            # ---------- DMA loads ----------
            # critical path (sync ring): small align weights, then padTb
            w4t_sb = big.tile([128, 2, 1], BF16, tag="w4t_sb")
            nc.sync.dma_start(w4t_sb[:].rearrange("p a b -> p (a b)"), w4t.ap())
            w3_sb = big.tile([128, 2, AH], BF16, tag="w3_sb")
            nc.sync.dma_start(w3_sb[:].rearrange("p a b -> p (a b)"), w3.ap())
            w2_sb = big.tile([128, 2, AH], BF16, tag="w2_sb")
            nc.sync.dma_start(w2_sb[:].rearrange("p a b -> p (a b)"), w2.ap())
            w1p_sb = big.tile([128, 2, HH], BF16, tag="w1p_sb")
            nc.sync.dma_start(w1p_sb[:].rearrange("p a b -> p (a b)"), w1p.ap())

            padTb_sb = big.tile([128, FO, BT], BF16, tag="padTb_sb")
            N_PCHUNK = 4
            for c in range(N_PCHUNK):
                fo0 = c * (FO // N_PCHUNK)
                fo1 = (c + 1) * (FO // N_PCHUNK)
                nc.sync.dma_start(
                    padTb_sb[:, fo0:fo1, :].rearrange("p a b -> p (a b)"),
                    padTb.ap()[:, fo0 * BT : fo1 * BT],
                )

            # weights on the scalar HWDGE ring (off the critical path)
            wl_sb = []
            for l in range(4):
                kt = FO if l == 0 else KT1
                t_ = big.tile([128, kt, 384], BF16, tag=f"wl{l}_sb")
                nc.scalar.dma_start(t_[:].rearrange("p a b -> p (a b)"), wl[l].ap())
                wl_sb.append(t_)

            b_sb = []
            for l in range(4):
                t_i = work.tile([128, 3], F32, tag=f"bih{l}")
                nc.scalar.dma_start(t_i[:], bih[l].ap())
                t_h = work.tile([128, 3], F32, tag=f"bhh{l}")
                nc.scalar.dma_start(t_h[:], bhh[l].ap())
                t_b = big.tile([128, 3], F32, tag=f"b{l}")
                nc.vector.tensor_add(t_b[:], t_i[:], t_h[:])
                b_sb.append(t_b)

            # ---------- compose m_P on PE (bf16 in, fp32 accum) ----------
            ps_u = pp.tile([128, 2], F32, tag="ps_u")
            for mt in range(2):
                for kt in range(2):
                    nc.tensor.matmul(
                        ps_u[:, mt : mt + 1],
                        w3_sb[:, kt, mt * 128 : (mt + 1) * 128],
                        w4t_sb[:, kt, :],
                        start=(kt == 0),
                        stop=(kt == 1),
                    )
            u1_sb = big.tile([128, 2], BF16, tag="u1_sb")
            nc.vector.tensor_copy(u1_sb[:], ps_u[:])

            ps_u2 = pp.tile([128, 2], F32, tag="ps_u")
            for mt in range(2):
                for kt in range(2):
                    nc.tensor.matmul(
                        ps_u2[:, mt : mt + 1],
                        w2_sb[:, kt, mt * 128 : (mt + 1) * 128],
                        u1_sb[:, kt : kt + 1],
                        start=(kt == 0),
                        stop=(kt == 1),
                    )
            u2_sb = big.tile([128, 2], BF16, tag="u2_sb")
            nc.vector.tensor_copy(u2_sb[:], ps_u2[:])

            ps_m = pp.tile([128, FO], F32, tag="ps_m")
            for mt in range(FO):
                for kt in range(2):
                    nc.tensor.matmul(
                        ps_m[:, mt : mt + 1],
                        w1p_sb[:, kt, mt * 128 : (mt + 1) * 128],
                        u2_sb[:, kt : kt + 1],
                        start=(kt == 0),
                        stop=(kt == 1),
                    )
            m_sb = big.tile([128, FO], BF16, tag="m_sb")
            nc.vector.tensor_copy(m_sb[:], ps_m[:])

            # ---------- e = m_P . padTb  -> [1, (b, t)] ----------
            e_ps = pp.tile([1, BT], F32, tag="e_ps")
            for fo in range(FO):
                nc.tensor.matmul(
                    e_ps[:],
                    m_sb[:, fo : fo + 1],
                    padTb_sb[:, fo, :],
                    start=(fo == 0),
                    stop=(fo == FO - 1),
                )

            # ---------- softmax over t (partition 0, fp32) ----------
            e_sb = big.tile([1, BT], F32, tag="e_sb")
            nc.scalar.activation(e_sb[:], e_ps[:], mybir.ActivationFunctionType.Copy)
            e3 = e_sb[:].rearrange("p (b t) -> p b t", b=BL)
            mx = big.tile([1, BL], F32, tag="mx")
            nc.vector.reduce_max(mx[:], e3, axis=mybir.AxisListType.X)
            ec = big.tile([1, BT], F32, tag="ec")
            nc.vector.tensor_sub(
                ec[:].rearrange("p (b t) -> p b t", b=BL),
                e3,
                mx[:].unsqueeze(2).broadcast_to([1, BL, T]),
            )
            p_sb = big.tile([1, BT], F32, tag="p_sb")
            nc.scalar.activation(p_sb[:], ec[:], mybir.ActivationFunctionType.Exp)
            p3 = p_sb[:].rearrange("p (b t) -> p b t", b=BL)
            s_sb = big.tile([1, BL], F32, tag="s_sb")
            nc.vector.reduce_sum(s_sb[:], p3, axis=mybir.AxisListType.X)
            rs = big.tile([1, BL], F32, tag="rs")
            nc.vector.reciprocal(rs[:], s_sb[:])
            a_16 = big.tile([1, BT], BF16, tag="a_16")
            nc.vector.tensor_mul(
                a_16[:].rearrange("p (b t) -> p b t", b=BL),
                p3,
                rs[:].unsqueeze(2).broadcast_to([1, BL, T]),
            )
            a_bc = big.tile([128, BT], BF16, tag="a_bc")
            nc.gpsimd.partition_broadcast(a_bc[:], a_16[:])

            # ---------- context^T, two halves, DVE + GpSimd split ----------
            ctJ = []
            for hh in range(2):
                ctx_sb = work.tile([128, FH, BL], F32, tag="ctx_sb")
                for fi in range(FH):
                    fo = hh * FH + fi
                    eng = nc.gpsimd if fi >= 4 else nc.vector
                    prod = work.tile([128, BT], BF16, tag=f"prod{fi >= 4}")
                    eng.tensor_mul(prod[:], padTb_sb[:, fo, :], a_bc[:])
                    nc.vector.tensor_reduce(
                        ctx_sb[:, fi, :],
                        prod[:].rearrange("p (b t) -> p b t", b=BL),
                        op=mybir.AluOpType.add,
                        axis=mybir.AxisListType.X,
                    )
                ctb_sb = work.tile([128, FH, BL], BF16, tag="ctb_sb")
                nc.vector.tensor_copy(ctb_sb[:], ctx_sb[:])
                nc.sync.dma_start(ct_in[hh].ap(), ctb_sb[:].rearrange("p a b -> p (a b)"))
                nc.gpsimd.collective_compute(
                    "AllGather",
                    mybir.AluOpType.bypass,
                    replica_groups=RG,
                    ins=[ct_in[hh].ap()],
                    outs=[ct_ag[hh].ap()],
                )
                # one clean [128, 64] load per rank block
                ctj = big.tile([128, NC, FH, BL], BF16, tag=f"ctJ{hh}")
                ag_r = ct_ag[hh].ap().rearrange(
                    "(j p) fb -> j p fb", j=NC, p=128
                )
                for j in range(NC):
                    nc.sync.dma_start(
                        ctj[:, j, :, :].rearrange("p a b -> p (a b)"), ag_r[j]
                    )
                ctJ.append(ctj)

            # ---------- 4-layer LSTM cell (i, g, o gates only) ----------
            rhs_sb = None  # layers >=1 use h tiles [128, kt, b]
            for l in range(4):
                kt_n = FO if l == 0 else KT1
                gates = []
                for m in range(3):
                    ps_g = pg.tile([128, B], F32, tag="gates")
                    for kt in range(kt_n):
                        if l == 0:
                            rhs = ctJ[kt // FH][:, :, kt % FH, :]
                        else:
                            rhs = rhs_sb[:, kt, :]
                        nc.tensor.matmul(
                            ps_g[:],
                            wl_sb[l][:, kt, m * 128 : (m + 1) * 128],
                            rhs,
                            start=(kt == 0),
                            stop=(kt == kt_n - 1),
                        )
                    gates.append(ps_g)
                sig_i = work.tile([128, B], F32, tag="sig_i")
                nc.scalar.activation(
                    sig_i[:], gates[0][:], mybir.ActivationFunctionType.Sigmoid,
                    bias=b_sb[l][:, 0:1],
                )
                tanh_g = work.tile([128, B], F32, tag="tanh_g")
                nc.scalar.activation(
                    tanh_g[:], gates[1][:], mybir.ActivationFunctionType.Tanh,
                    bias=b_sb[l][:, 1:2],
                )
                c_t = work.tile([128, B], F32, tag="c_t")
                nc.vector.tensor_mul(c_t[:], sig_i[:], tanh_g[:])
                tanh_c = work.tile([128, B], F32, tag="tanh_c")
                nc.scalar.activation(
                    tanh_c[:], c_t[:], mybir.ActivationFunctionType.Tanh
                )
                sig_o = work.tile([128, B], F32, tag="sig_o")
                nc.scalar.activation(
                    sig_o[:], gates[2][:], mybir.ActivationFunctionType.Sigmoid,
                    bias=b_sb[l][:, 2:3],
                )
                h_sl = work.tile([128, B], F32, tag="h_sl")
                nc.vector.tensor_mul(h_sl[:], sig_o[:], tanh_c[:])

                if l < 3:
                    h_slb = work.tile([128, B], BF16, tag="h_slb")
                    nc.vector.tensor_copy(h_slb[:], h_sl[:])
                    nc.sync.dma_start(h_in[l].ap(), h_slb[:])
                    nc.gpsimd.collective_compute(
                        "AllGather",
                        mybir.AluOpType.bypass,
                        replica_groups=RG,
                        ins=[h_in[l].ap()],
                        outs=[h_ag[l].ap()],
                    )
                    nxt = big.tile([128, KT1, B], BF16, tag=f"h{l}_sb")
                    nc.sync.dma_start(
                        nxt[:], h_ag[l].ap().rearrange("(kt p) b -> p kt b", p=128)
                    )
                    rhs_sb = nxt
                else:
                    nc.sync.dma_start(out.ap(), h_sl[:])

    nc.compile()
    return nc


def _prep_inputs(padded, align_ws, w_ih, b_ih, b_hh):
    """Build the 8 per-core input maps (host-side sharding / layout only)."""
    padded = np.asarray(padded, dtype=np.float32)
    w1 = np.asarray(align_ws[0], dtype=np.float32)
    shared = {
        "w1p": _pmajor(np.ascontiguousarray(w1[:, H:]).astype(NPBF)),
        "w2": _pmajor(np.asarray(align_ws[1], dtype=np.float32).astype(NPBF)),
        "w3": _pmajor(np.asarray(align_ws[2], dtype=np.float32).astype(NPBF)),
        "w4t": _pmajor(np.asarray(align_ws[3], dtype=np.float32).reshape(AH, 1).astype(NPBF)),
    }
    in_maps = []
    for k in range(NC):
        m = dict(shared)
        # padded^T for batches [8k, 8k+8): [2048, 8, 50] -> [2048, 400]
        sl = padded[:, k * BL : (k + 1) * BL, :]          # [50, 8, 2048]
        pT = np.ascontiguousarray(sl.transpose(2, 1, 0)).reshape(HH, BT)
        m["padTb"] = _pmajor(pT.astype(NPBF))
        for l in range(4):
            w = np.asarray(w_ih[l], dtype=np.float32)
            rows = np.concatenate(
                [w[g * H + k * 128 : g * H + (k + 1) * 128, :] for g in (0, 2, 3)],
                axis=0,
            )                                              # [384, fin] i,g,o rows
            m[f"wl{l}"] = _pmajor(np.ascontiguousarray(rows.T).astype(NPBF))
            for name, b in (("bih", b_ih[l]), ("bhh", b_hh[l])):
                bb = np.asarray(b, dtype=np.float32)
                bsel = np.stack(
                    [bb[g * H + k * 128 : g * H + (k + 1) * 128] for g in (0, 2, 3)],
                    axis=1,
                )                                          # [128, 3]
                m[f"{name}{l}"] = np.ascontiguousarray(bsel)
        in_maps.append(m)
    return in_maps


def kernel(padded, align_ws, align_bs, w_ih, w_hh, b_ih, b_hh):
    # align_bs and w_hh are mathematically inert: the MLP biases add a
    # t-invariant constant to e (cancels in softmax); w_hh multiplies the
    # zero initial LSTM state.
    if "nc" not in _CACHE:
        _CACHE["nc"] = _build()
    nc = _CACHE["nc"]
    in_maps = _prep_inputs(padded, align_ws, w_ih, b_ih, b_hh)
    res = run_bass_kernel_spmd(nc, in_maps, list(range(NC)))
    hT = np.concatenate([res.results[k]["out"] for k in range(NC)], axis=0)  # [1024, 64]
    h = hT.T                                                                  # [64, 1024]
    return np.ascontiguousarray(
        np.broadcast_to(h[None, :, :], (T, B, H))
    ).astype(np.float32)
